# revision 31
# baseline (speedup 1.0000x reference)
"""Multi-head causal self-attention on 8 Trainium2 NeuronCores.

Problem: B=4, T=2048, D=1024, H=16 heads, Hd=64. fp32.
Sharding: core c handles batch b = c//2 and head-group g = c%2 (8 heads,
512 channels). Each core computes a partial output (its head-group's
contribution to x @ Wo); the host sums head-group pairs and adds bo.

Per-core dataflow (host sends x already transposed, so there is no
on-chip transpose anywhere):
  xT   [D=1024, T]   16 SBUF tiles [128, 1024] (fine-grained DMA deps)
  Q^T  [C=512, T]    = matmul(lhsT=Wq chunk, rhs=x^T)   (head h at partitions
  K^T  [C=512, T]      64*(h%2) .. of chunk h//2)
  V'   [T, 8*65]     = matmul(lhsT=x^T chunk, rhs=Wv), per head [V(64) | 1]
  S^T  [k,q]         = matmul(lhsT=K^T block, rhs=Q^T span)  (k on partitions)
  E = exp(S^T/8)     on ScalarE PSUM->SBUF; DVE staircase-mask multiply on
                     only the leading 128 cols of diagonal blocks
  ctx' [65, q]       = matmul(lhsT=V' block, rhs=E)  accumulated over k blocks
                       row 64 = softmax denominator (ones-column trick)
  normalize          denom row -> DRAM bounce broadcast to partitions 0:64
                     -> reciprocal_approx_fast -> DVE mult; head B shifted to
                     partitions 64:128 by one small DMA.  (The final chunk
                     instead broadcasts with a PE ones-matmul over the zden
                     tile, since the tail's output DMAs would delay the
                     bounce on the sync queue.)
  out  [T, D]        = matmul(lhsT=ctx^T chunk, rhs=Wo chunk), DMA out

Scheduling (engine queues are in-order, so emission order is the
schedule):
- k-block kb's S matmuls + exp are emitted BEFORE k-block kb-1's ctx
  matmuls, so the PE streams S(kb) while ScalarE runs exp(kb-1) instead
  of ping-ponging (exp is the phase-C bottleneck at ~1.16us/k-block).
- Span-outer / head-pair-inner loop; V'(4..15) and the previous spans'
  output-projection groups are emitted as fillers inside later chunks'
  k-loops, each scheduled at least one chunk after the normalize whose
  ctxT rows it reads (ctxT dependency tracking is per-tile).
- PSUM: st pool (2x2 banks: S tiles + short-lived V'/QK/out-proj
  groups) + cs pool (2x2 banks: ctx accumulators).  With bufs=2, PSUM
  allocation k waits on allocation k-2's release, which constrains
  where filler groups may allocate (see comments).
- Tail: D(2,3) and the hp<3 partials of D(3,0/1) are emitted before the
  final normalize so the PE is busy while its chain completes.
Causality: only k-blocks with k0 <= q_span_end are computed; the <=4
diagonal blocks per span get the leading-128-column staircase mask.
"""

import sys

for _p in ("/opt/trn_rl_repo", "/root/.axon_site/_ro/trn_rl_repo"):
    if _p not in sys.path:
        sys.path.append(_p)

import numpy as np

import concourse.bacc as bacc
import concourse.mybir as mybir
import concourse.tile as tile
from concourse.bass_utils import run_bass_kernel_spmd

FP32 = mybir.dt.float32
BF16 = mybir.dt.bfloat16
P = 128
T = 2048  # sequence length
D = 1024  # model dim
C = 512   # channels per core (8 heads)
H = 8     # heads per core
HD = 64   # head dim
N_CORES = 8
NSPAN = 4          # q spans of 512
SPAN = 512
NKB = 16           # k blocks of 128

_program = None


def _build():
    nc = bacc.Bacc()
    xt_d = nc.declare_dram_parameter("xt", [D, T], BF16, isOutput=False)
    wq_d = nc.declare_dram_parameter("wq", [D, C], BF16, isOutput=False)
    wk_d = nc.declare_dram_parameter("wk", [D, C], BF16, isOutput=False)
    wv_d = nc.declare_dram_parameter("wv", [D, C], BF16, isOutput=False)
    wo_d = nc.declare_dram_parameter("wo", [C, D], BF16, isOutput=False)
    mask_d = nc.declare_dram_parameter("mask", [P, P], BF16, isOutput=False)
    out_d = nc.declare_dram_parameter("out", [T, D], FP32, isOutput=True)

    Exp = mybir.ActivationFunctionType.Exp

    def copy_px(idx, dst, src):
        # alternate PSUM->SBUF copies between ScalarE and VectorE
        if idx % 2 == 0:
            nc.scalar.copy(dst, src)
        else:
            nc.vector.tensor_copy(dst, src)

    from contextlib import ExitStack

    with tile.TileContext(nc) as tc, ExitStack() as persist:
        const_pool = persist.enter_context(tc.tile_pool(name="const", bufs=1))
        xt_pool = persist.enter_context(tc.tile_pool(name="xt", bufs=1))
        qkt_pool = persist.enter_context(tc.tile_pool(name="qkt", bufs=1))
        vp_pool = persist.enter_context(tc.tile_pool(name="vp", bufs=1))
        persist_w = persist.enter_context(tc.tile_pool(name="pw", bufs=1))
        ctxT_pool = persist.enter_context(tc.tile_pool(name="ctxT", bufs=1))

        mask_sb = const_pool.tile([P, P], BF16, tag="mask")
        ones_sb = const_pool.tile([P, HD], BF16, tag="ones")
        # zden: all-zero except row 64, used by the final chunk's PE
        # ones-broadcast normalize (the DRAM bounce would queue behind the
        # tail's output DMAs on the sync queue)
        zden = const_pool.tile([P, 1024], BF16, tag="zden")
        nc.gpsimd.memset(ones_sb[:], 1.0)
        nc.gpsimd.memset(zden[:], 0.0)

        # xT as 16 independent tiles: [j-chunk of D][half of T]
        xt = [[xt_pool.tile([P, T // 2], BF16, tag=f"xt{j}_{h}",
                            name=f"xt{j}_{h}") for h in range(2)]
              for j in range(8)]
        qt = [qkt_pool.tile([P, T], BF16, tag=f"qt{i}", name=f"qt{i}") for i in range(4)]
        kt = [qkt_pool.tile([P, T], BF16, tag=f"kt{i}", name=f"kt{i}") for i in range(4)]
        vp = [vp_pool.tile([P, H * 65], BF16, tag=f"vp{t}", name=f"vp{t}") for t in range(NKB)]
        ctxT = [ctxT_pool.tile([P, T], BF16, tag=f"ct{i}", name=f"ct{i}")
                for i in range(4)]

        wv_sb = persist_w.tile([P, 8, C], BF16, tag="wv")
        wq_sb = persist_w.tile([P, 8, C], BF16, tag="wq")
        wk_sb = persist_w.tile([P, 8, C], BF16, tag="wk")
        wo_sb = persist_w.tile([P, 4, D], BF16, tag="wo")

        # --- input DMAs, ordered so the V' pipeline starts ASAP ---------
        # (DMA issue on the sync queue is ~0.6us each, so the V'-critical
        # inputs use as few issues as possible)
        nc.sync.dma_start(wv_sb[:], wv_d.rearrange("(o p) c -> p o c", p=P))
        for j in range(8):
            nc.sync.dma_start(xt[j][0][:], xt_d[j * P:(j + 1) * P, 0:T // 2])
        nc.sync.dma_start(wq_sb[:], wq_d.rearrange("(o p) c -> p o c", p=P))
        nc.sync.dma_start(mask_sb[:], mask_d[:])
        for j in range(8):
            nc.sync.dma_start(xt[j][1][:], xt_d[j * P:(j + 1) * P, T // 2:T])
        nc.sync.dma_start(wk_sb[:], wk_d.rearrange("(o p) c -> p o c", p=P))
        nc.sync.dma_start(wo_sb[:], wo_d.rearrange("(o p) d -> p o d", p=P))
        for t in range(NKB):
            nc.gpsimd.memset(vp[t][:], 1.0)

        def xts(j, t0, w):
            # xT slice [128, w] at t-offset t0 from the fine-grained tiles
            h = t0 // (T // 2)
            assert (t0 + w - 1) // (T // 2) == h
            return xt[j][h][:, t0 - h * (T // 2): t0 + w - h * (T // 2)]

        ci = 0
        with (
            tc.tile_pool(name="stps", bufs=2, space="PSUM") as st_pool,
            tc.tile_pool(name="csps", bufs=2, space="PSUM") as cs_pool,
            tc.tile_pool(name="epool", bufs=6) as e_pool,
            tc.tile_pool(name="npool", bufs=3) as n_pool,
            tc.tile_pool(name="osb", bufs=4) as o_pool,
            tc.tile_pool(name="rdram", bufs=2, space="DRAM") as rd_pool,
        ):
            # All PSUM comes from two 2-bank pools: st (S-score tiles, and
            # short-lived V'/QK/out-projection group tiles) and cs (ctx
            # accumulators only; out-projection after phase C). Every st
            # allocation's awaited release (exp read / copy) is emitted in
            # the same block, so the in-order queues can never deadlock.
            def emit_v(t, alt=False):
                nonlocal ci
                ps = st_pool.tile([P, 1024], FP32, tag="st")
                for j in range(8):
                    nc.tensor.matmul(ps[:, 0:C], xts(j, t * P, P), wv_sb[:, j, :],
                                     start=(j == 0), stop=(j == 7))
                dst = vp[t].rearrange("p (h e) -> p h e", e=65)[:, :, 0:64]
                src = ps.rearrange("p (h e) -> p h e", e=64)[:, 0:8, :]
                if alt:
                    copy_px(ci, dst, src)
                    ci += 1
                else:
                    nc.vector.tensor_copy(dst, src)

            def emit_qk(dst, wsb, hp, s):
                nonlocal ci
                ps = st_pool.tile([P, 1024], FP32, tag="st")
                for j in range(8):
                    nc.tensor.matmul(ps[:, 0:SPAN],
                                     wsb[:, j, hp * P:(hp + 1) * P],
                                     xts(j, s * SPAN, SPAN),
                                     start=(j == 0), stop=(j == 7))
                copy_px(ci, dst[hp][:, s * SPAN:(s + 1) * SPAN], ps[:, 0:SPAN])
                ci += 1

            def emit_norm(hp, s, cs):
                # normalize: rows 0..63 / row 64 (ones-column rowsum).
                # Denominator row is bounced through DRAM to broadcast it
                # across partitions 0:64 (no PE or PSUM involvement), then
                # approx-reciprocal at base partition 0 and DVE multiply.
                qsl = slice(s * SPAN, (s + 1) * SPAN)
                rs = n_pool.tile([P, 1024], FP32, tag="rs")
                nc.vector.tensor_copy(rs[64:65, :], cs[64:65, :])
                rd = rd_pool.tile([1024], FP32, tag="rd")
                nc.sync.dma_start(rd[None, :], rs[64:65, :])
                rb = n_pool.tile([P, 1024], FP32, tag="rb")
                nc.sync.dma_start(rb[0:64, :],
                                  rd[None, :].to_broadcast((64, 1024)))
                rr = n_pool.tile([P, 1024], FP32, tag="rr")
                nc.vector.reciprocal_approx_fast(rr[0:64, :], rb[0:64, :])
                nc.vector.tensor_mul(ctxT[hp][0:64, qsl],
                                     cs[0:64, 0:512], rr[0:64, 0:512])
                tmpB = n_pool.tile([P, SPAN], BF16, tag="tmpB")
                nc.vector.tensor_mul(tmpB[0:64, :],
                                     cs[0:64, 512:1024], rr[0:64, 512:1024])
                nc.sync.dma_start(ctxT[hp][64:128, qsl], tmpB[0:64, :])

            def emit_d(s, qb, nhs, pool, alt=False):
                # output projection for q-block qb of span s, D-halves nhs;
                # contraction over all 4 head-pairs
                nonlocal ci
                qg = 4 * s + qb
                ps = pool.tile([P, 1024], FP32,
                               tag="st" if pool is st_pool else "cs")
                for nh in nhs:
                    for hp in range(4):
                        nc.tensor.matmul(
                            ps[:, nh * SPAN:(nh + 1) * SPAN],
                            ctxT[hp][:, qg * P:(qg + 1) * P],
                            wo_sb[:, hp, nh * SPAN:(nh + 1) * SPAN],
                            start=(hp == 0), stop=(hp == 3))
                c0, w = nhs[0] * SPAN, len(nhs) * SPAN
                ot = o_pool.tile([P, 1024], FP32, tag="osb")
                if alt and w == 1024:
                    # split copy across both engines; each half's output
                    # DMA issues as soon as its own copy lands
                    nc.scalar.copy(ot[:, 0:SPAN], ps[:, 0:SPAN])
                    nc.sync.dma_start(
                        out_d[qg * P:(qg + 1) * P, 0:SPAN], ot[:, 0:SPAN])
                    nc.vector.tensor_copy(ot[:, SPAN:1024], ps[:, SPAN:1024])
                    nc.sync.dma_start(
                        out_d[qg * P:(qg + 1) * P, SPAN:1024],
                        ot[:, SPAN:1024])
                    return
                if alt:
                    copy_px(ci, ot[:, c0:c0 + w], ps[:, c0:c0 + w])
                    ci += 1
                else:
                    nc.vector.tensor_copy(ot[:, c0:c0 + w], ps[:, c0:c0 + w])
                nc.sync.dma_start(
                    out_d[qg * P:(qg + 1) * P, c0:c0 + w], ot[:, c0:c0 + w])

            def emit_c(hp, s, fillers):
                # Software-pipelined emission: k-block kb's S matmuls and
                # exp are emitted BEFORE k-block kb-1's ctx matmuls, so the
                # in-order PE queue streams S(kb) while ScalarE runs
                # exp(kb-1). One filler (normalize of the previous chunk,
                # then V'/out-projection groups) is popped per k-block.
                hA, hB = 2 * hp, 2 * hp + 1
                cs = None
                nkb = 4 * s + 4
                pend = None          # (e, kb, d) awaiting ctx emission

                def emit_ctx(e, kb, d):
                    nonlocal cs
                    if cs is None:
                        cs = cs_pool.tile([P, 1024], FP32, tag="cs")
                    w = SPAN - 128 * d
                    co = 128 * d
                    nc.tensor.matmul(cs[0:65, co:SPAN],
                                     vp[kb][:, hA * 65:(hA + 1) * 65],
                                     e[:, 0:w],
                                     start=(kb == 0), stop=(kb == nkb - 1))
                    nc.tensor.matmul(cs[0:65, 512 + co:1024],
                                     vp[kb][:, hB * 65:(hB + 1) * 65],
                                     e[:, 512:512 + w],
                                     start=(kb == 0), stop=(kb == nkb - 1))

                for kb in range(nkb):
                    ksl = slice(kb * P, (kb + 1) * P)
                    d = max(0, kb - 4 * s)      # diagonal offset 0..3
                    q0 = s * SPAN + 128 * d     # valid q start
                    w = SPAN - 128 * d          # valid width
                    qsl = slice(q0, (s + 1) * SPAN)
                    st = st_pool.tile([P, 1024], FP32, tag="st")
                    st3 = st.rearrange("p (b q) -> p b q", b=2)[:, :, 0:w]
                    nc.tensor.matmul(st[:, 0:w], kt[hp][0:64, ksl],
                                     qt[hp][0:64, qsl],
                                     start=True, stop=True)
                    nc.tensor.matmul(st[:, 512:512 + w], kt[hp][64:128, ksl],
                                     qt[hp][64:128, qsl],
                                     start=True, stop=True)
                    e = e_pool.tile([P, 1024], BF16, tag="e")
                    e3 = e.rearrange("p (b q) -> p b q", b=2)[:, :, 0:w]
                    nc.scalar.activation(e3, st3, Exp, scale=0.125)
                    if kb >= 4 * s:
                        # staircase lives in the leading 128 cols only
                        e128 = e.rearrange("p (b q) -> p b q", b=2)[:, :, 0:P]
                        nc.vector.tensor_mul(
                            e128, e128,
                            mask_sb[:, None, :].to_broadcast((P, 2, P)))
                    if pend is not None:
                        if fillers:
                            fillers.pop(0)()
                        emit_ctx(*pend)
                    pend = (e, kb, d)
                while fillers:
                    fillers.pop(0)()
                emit_ctx(*pend)
                return cs

            # ---- Phase B runway: V'(0..3) and head-pair 0's Q^T/K^T -----
            # (the other head-pairs' QK groups are emitted as fillers
            # inside span-0's chunks, giving the PE independent work while
            # ScalarE chews span-0's exps, and starting exp ~70us earlier)
            for t in range(4):
                emit_v(t, alt=True)
            for s in range(NSPAN):
                emit_qk(qt, wq_sb, 0, s)
                emit_qk(kt, wk_sb, 0, s)

            # ---- Phase C with interleaved V' / output projection --------
            # filler schedule (2 PE filler groups per chunk):
            #   span 0: V'(4..11)    span 1: V'(12..15) + D(0) full groups
            #   span 2: D(1) halves  span 3: D(2) halves
            # Each out-projection group D(s, qb) is scheduled at least one
            # full chunk after norm(3, s) so the DRAM-bounce normalize
            # chain (~6us) never stalls its ctxT reads.
            prev = None
            for s in range(NSPAN):
                for hp in range(4):
                    fillers = []
                    if prev is not None:
                        ph, ps_, pc = prev
                        fillers.append(
                            lambda ph=ph, ps_=ps_, pc=pc:
                                emit_norm(ph, ps_, pc))
                    if s == 0:
                        if hp < 3:
                            for s2 in range(NSPAN):
                                fillers.append(
                                    lambda s2=s2, hp=hp:
                                        emit_qk(qt, wq_sb, hp + 1, s2))
                                fillers.append(
                                    lambda s2=s2, hp=hp:
                                        emit_qk(kt, wk_sb, hp + 1, s2))
                        fillers.append(lambda t=4 + 2 * hp: emit_v(t))
                        fillers.append(lambda t=5 + 2 * hp: emit_v(t))
                    elif s == 1:
                        fillers.append(lambda t=12 + hp: emit_v(t))
                        if hp > 0:
                            fillers.append(
                                lambda hp=hp: emit_d(0, hp - 1, (0, 1), st_pool))
                    else:
                        if hp == 0:
                            fillers.append(
                                lambda s=s: emit_d(s - 2, 3, (0, 1), st_pool))
                        else:
                            fillers.append(
                                lambda s=s, hp=hp:
                                    emit_d(s - 1, hp - 1, (0, 1), st_pool))
                    cs = emit_c(hp, s, fillers)
                    prev = (hp, s, cs)
            # Tail: D(2,3) and the hp<3 partial sums of D(3,0/1) are
            # emitted BEFORE the final normalize, so the PE chews them
            # while the DRAM-bounce chain for chunk (3,3) completes
            # (ctxT tile deps are coarse, so anything reading ctxT[*]
            # emitted after that norm waits for it).
            fhp, fs, fcs = prev
            fqsl = slice(fs * SPAN, (fs + 1) * SPAN)
            nc.vector.tensor_copy(zden[64:65, :], fcs[64:65, :])
            fbc = st_pool.tile([P, 1024], FP32, tag="st")
            nc.tensor.matmul(fbc[0:64, 0:512], ones_sb[64:128, 0:64],
                             zden[64:128, 0:512], start=True, stop=True)
            nc.tensor.matmul(fbc[0:64, 512:1024], ones_sb[64:128, 0:64],
                             zden[64:128, 512:1024], start=True, stop=True)
            frr = n_pool.tile([P, 1024], FP32, tag="rr")
            nc.vector.reciprocal_approx_fast(frr[0:64, :], fbc[0:64, :])
            nc.vector.tensor_mul(ctxT[fhp][0:64, fqsl],
                                 fcs[0:64, 0:512], frr[0:64, 0:512])
            ftmp = n_pool.tile([P, SPAN], BF16, tag="tmpB")
            nc.vector.tensor_mul(ftmp[0:64, :],
                                 fcs[0:64, 512:1024], frr[0:64, 512:1024])
            nc.sync.dma_start(ctxT[fhp][64:128, fqsl], ftmp[0:64, :])
            emit_d(NSPAN - 2, 3, (0, 1), st_pool, alt=True)
            pd = []
            for qb in (0, 1):
                ps3 = cs_pool.tile([P, 1024], FP32, tag="cs")
                for nh in (0, 1):
                    for hp in range(3):
                        nc.tensor.matmul(
                            ps3[:, nh * SPAN:(nh + 1) * SPAN],
                            ctxT[hp][:, (12 + qb) * P:(13 + qb) * P],
                            wo_sb[:, hp, nh * SPAN:(nh + 1) * SPAN],
                            start=(hp == 0), stop=False)
                pd.append(ps3)
            for qb in (0, 1):
                ps3 = pd[qb]
                for nh in (0, 1):
                    nc.tensor.matmul(
                        ps3[:, nh * SPAN:(nh + 1) * SPAN],
                        ctxT[3][:, (12 + qb) * P:(13 + qb) * P],
                        wo_sb[:, 3, nh * SPAN:(nh + 1) * SPAN],
                        start=False, stop=True)
                ot = o_pool.tile([P, 1024], FP32, tag="osb")
                nc.scalar.copy(ot[:, 0:SPAN], ps3[:, 0:SPAN])
                nc.sync.dma_start(
                    out_d[(12 + qb) * P:(13 + qb) * P, 0:SPAN], ot[:, 0:SPAN])
                nc.vector.tensor_copy(ot[:, SPAN:1024], ps3[:, SPAN:1024])
                nc.sync.dma_start(
                    out_d[(12 + qb) * P:(13 + qb) * P, SPAN:1024],
                    ot[:, SPAN:1024])
            for qb in (2, 3):
                emit_d(NSPAN - 1, qb, (0, 1), cs_pool, alt=True)

    nc.compile()
    return nc


def _get_program():
    global _program
    if _program is None:
        _program = _build()
    return _program


def _make_mask():
    import ml_dtypes
    j = np.arange(P)[None, :]
    k = np.arange(P)[:, None]
    return np.where(j >= k, 1.0, 0.0).astype(ml_dtypes.bfloat16)


def _make_in_maps(x, Wq, Wk, Wv, Wo):
    import ml_dtypes
    bf16 = ml_dtypes.bfloat16
    mask = _make_mask()
    wq = np.asarray(Wq, np.float32).astype(bf16)
    wk = np.asarray(Wk, np.float32).astype(bf16)
    wv = np.asarray(Wv, np.float32).astype(bf16)
    wo = np.asarray(Wo, np.float32).astype(bf16)
    # x transposed on host: one [D, T] array per batch, shared by 2 cores
    xts = [np.ascontiguousarray(np.asarray(x[b], np.float32).astype(bf16).T)
           for b in range(x.shape[0])]
    in_maps = []
    for c in range(N_CORES):
        b, g = c // 2, c % 2
        cols = slice(g * C, (g + 1) * C)
        in_maps.append({
            "xt": xts[b],
            "wq": np.ascontiguousarray(wq[:, cols]),
            "wk": np.ascontiguousarray(wk[:, cols]),
            "wv": np.ascontiguousarray(wv[:, cols]),
            "wo": np.ascontiguousarray(wo[cols, :]),
            "mask": mask,
        })
    return in_maps


def _combine(results, bo, B):
    out = np.empty((B, T, D), dtype=np.float32)
    bo = np.asarray(bo, dtype=np.float32)
    for b in range(B):
        out[b] = results[2 * b]["out"] + results[2 * b + 1]["out"] + bo
    return out


def kernel(x, Wq, Wk, Wv, Wo, bo):
    x = np.asarray(x)
    nc = _get_program()
    in_maps = _make_in_maps(x, Wq, Wk, Wv, Wo)
    res = run_bass_kernel_spmd(nc, in_maps, core_ids=list(range(N_CORES)))
    return _combine(res.results, bo, x.shape[0])


def kernel_traced(x, Wq, Wk, Wv, Wo, bo):
    """Like kernel() but also returns the BassKernelResults (with
    exec_time_ns when NTFF tracing is available)."""
    x = np.asarray(x)
    nc = _get_program()
    in_maps = _make_in_maps(x, Wq, Wk, Wv, Wo)
    res = run_bass_kernel_spmd(nc, in_maps, core_ids=list(range(N_CORES)),
                               trace=True)
    return _combine(res.results, bo, x.shape[0]), res


# revision 34
# speedup vs baseline: 1.0173x; 1.0173x over previous
"""Multi-head causal self-attention on 8 Trainium2 NeuronCores.

Problem: B=4, T=2048, D=1024, H=16 heads, Hd=64. fp32.
Sharding: core c handles batch b = c//2 and head-group g = c%2 (8 heads,
512 channels). Each core computes a partial output (its head-group's
contribution to x @ Wo); the host sums head-group pairs and adds bo.

Per-core dataflow (host sends x already transposed, so there is no
on-chip transpose anywhere):
  xT   [D=1024, T]   16 SBUF tiles [128, 1024] (fine-grained DMA deps)
  Q^T  [C=512, T]    = matmul(lhsT=Wq chunk, rhs=x^T)   (head h at partitions
  K^T  [C=512, T]      64*(h%2) .. of chunk h//2)
  V'   [T, 8*65]     = matmul(lhsT=x^T chunk, rhs=Wv), per head [V(64) | 1]
  S^T  [k,q]         = matmul(lhsT=K^T block, rhs=Q^T span)  (k on partitions)
  E = exp(S^T/8)     on ScalarE PSUM->SBUF; DVE staircase-mask multiply on
                     only the leading 128 cols of diagonal blocks
  ctx' [65, q]       = matmul(lhsT=V' block, rhs=E)  accumulated over k blocks
                       row 64 = softmax denominator (ones-column trick)
  normalize          denom row -> DRAM bounce broadcast to partitions 0:64
                     -> reciprocal_approx_fast -> DVE mult; head B shifted to
                     partitions 64:128 by one small DMA.  (The final chunk
                     instead broadcasts with a PE ones-matmul over the zden
                     tile, since the tail's output DMAs would delay the
                     bounce on the sync queue.)
  out  [T, D]        = matmul(lhsT=ctx^T chunk, rhs=Wo chunk), DMA out

Scheduling (engine queues are in-order, so emission order is the
schedule):
- k-block kb's S matmuls + exp are emitted BEFORE k-block kb-1's ctx
  matmuls, so the PE streams S(kb) while ScalarE runs exp(kb-1) instead
  of ping-ponging (exp is the phase-C bottleneck at ~1.16us/k-block).
- Span-outer / head-pair-inner loop; V'(4..15) and the previous spans'
  output-projection groups are emitted as fillers inside later chunks'
  k-loops, each scheduled at least one chunk after the normalize whose
  ctxT rows it reads (ctxT dependency tracking is per-tile).
- PSUM: st pool (2x2 banks: S tiles + short-lived V'/QK/out-proj
  groups) + cs pool (2x2 banks: ctx accumulators).  With bufs=2, PSUM
  allocation k waits on allocation k-2's release, which constrains
  where filler groups may allocate (see comments).
- Tail: D(2,3) and the hp<3 partials of D(3,0/1) are emitted before the
  final normalize so the PE is busy while its chain completes.
Causality: only k-blocks with k0 <= q_span_end are computed; the <=4
diagonal blocks per span get the leading-128-column staircase mask.
"""

import sys

for _p in ("/opt/trn_rl_repo", "/root/.axon_site/_ro/trn_rl_repo"):
    if _p not in sys.path:
        sys.path.append(_p)

import numpy as np

import concourse.bacc as bacc
import concourse.mybir as mybir
import concourse.tile as tile
from concourse.bass_utils import run_bass_kernel_spmd

FP32 = mybir.dt.float32
BF16 = mybir.dt.bfloat16
P = 128
T = 2048  # sequence length
D = 1024  # model dim
C = 512   # channels per core (8 heads)
H = 8     # heads per core
HD = 64   # head dim
N_CORES = 8
NSPAN = 4          # q spans of 512
SPAN = 512
NKB = 16           # k blocks of 128

_program = None


def _build():
    nc = bacc.Bacc()
    xt_d = nc.declare_dram_parameter("xt", [D, T], BF16, isOutput=False)
    wq_d = nc.declare_dram_parameter("wq", [D, C], BF16, isOutput=False)
    wk_d = nc.declare_dram_parameter("wk", [D, C], BF16, isOutput=False)
    wv_d = nc.declare_dram_parameter("wv", [D, C], BF16, isOutput=False)
    wo_d = nc.declare_dram_parameter("wo", [C, D], BF16, isOutput=False)
    mask_d = nc.declare_dram_parameter("mask", [P, P], BF16, isOutput=False)
    out_d = nc.declare_dram_parameter("out", [T, D], FP32, isOutput=True)

    Exp = mybir.ActivationFunctionType.Exp

    def copy_px(idx, dst, src):
        # alternate PSUM->SBUF copies between ScalarE and VectorE
        if idx % 2 == 0:
            nc.scalar.copy(dst, src)
        else:
            nc.vector.tensor_copy(dst, src)

    from contextlib import ExitStack

    with tile.TileContext(nc) as tc, ExitStack() as persist:
        const_pool = persist.enter_context(tc.tile_pool(name="const", bufs=1))
        xt_pool = persist.enter_context(tc.tile_pool(name="xt", bufs=1))
        qkt_pool = persist.enter_context(tc.tile_pool(name="qkt", bufs=1))
        vp_pool = persist.enter_context(tc.tile_pool(name="vp", bufs=1))
        persist_w = persist.enter_context(tc.tile_pool(name="pw", bufs=1))
        ctxT_pool = persist.enter_context(tc.tile_pool(name="ctxT", bufs=1))

        mask_sb = const_pool.tile([P, P], BF16, tag="mask")
        ones_sb = const_pool.tile([P, HD], BF16, tag="ones")
        # zden: all-zero except row 64, used by the final chunk's PE
        # ones-broadcast normalize (the DRAM bounce would queue behind the
        # tail's output DMAs on the sync queue)
        zden = const_pool.tile([P, 1024], BF16, tag="zden")
        nc.gpsimd.memset(ones_sb[:], 1.0)
        nc.gpsimd.memset(zden[:], 0.0)

        # xT as 16 independent tiles: [j-chunk of D][half of T]
        xt = [[xt_pool.tile([P, T // 2], BF16, tag=f"xt{j}_{h}",
                            name=f"xt{j}_{h}") for h in range(2)]
              for j in range(8)]
        qt = [qkt_pool.tile([P, T], BF16, tag=f"qt{i}", name=f"qt{i}") for i in range(4)]
        kt = [qkt_pool.tile([P, T], BF16, tag=f"kt{i}", name=f"kt{i}") for i in range(4)]
        vp = [vp_pool.tile([P, H * 65], BF16, tag=f"vp{t}", name=f"vp{t}") for t in range(NKB)]
        ctxT = [ctxT_pool.tile([P, T], BF16, tag=f"ct{i}", name=f"ct{i}")
                for i in range(4)]

        wv_sb = persist_w.tile([P, 8, C], BF16, tag="wv")
        wq_sb = persist_w.tile([P, 8, C], BF16, tag="wq")
        wk_sb = persist_w.tile([P, 8, C], BF16, tag="wk")
        wo_sb = persist_w.tile([P, 4, D], BF16, tag="wo")

        # --- input DMAs, ordered so the V' pipeline starts ASAP ---------
        # (DMA issue on the sync queue is ~0.6us each, so the V'-critical
        # inputs use as few issues as possible)
        nc.sync.dma_start(wv_sb[:], wv_d.rearrange("(o p) c -> p o c", p=P))
        for j in range(8):
            nc.sync.dma_start(xt[j][0][:], xt_d[j * P:(j + 1) * P, 0:T // 2])
        nc.sync.dma_start(wq_sb[:], wq_d.rearrange("(o p) c -> p o c", p=P))
        nc.sync.dma_start(mask_sb[:], mask_d[:])
        for j in range(8):
            nc.sync.dma_start(xt[j][1][:], xt_d[j * P:(j + 1) * P, T // 2:T])
        nc.sync.dma_start(wk_sb[:], wk_d.rearrange("(o p) c -> p o c", p=P))
        nc.sync.dma_start(wo_sb[:], wo_d.rearrange("(o p) d -> p o d", p=P))
        for t in range(NKB):
            nc.gpsimd.memset(vp[t][:], 1.0)

        def xts(j, t0, w):
            # xT slice [128, w] at t-offset t0 from the fine-grained tiles
            h = t0 // (T // 2)
            assert (t0 + w - 1) // (T // 2) == h
            return xt[j][h][:, t0 - h * (T // 2): t0 + w - h * (T // 2)]

        ci = 0
        with (
            tc.tile_pool(name="stps", bufs=2, space="PSUM") as st_pool,
            tc.tile_pool(name="csps", bufs=2, space="PSUM") as cs_pool,
            tc.tile_pool(name="epool", bufs=6) as e_pool,
            tc.tile_pool(name="npool", bufs=3) as n_pool,
            tc.tile_pool(name="osb", bufs=4) as o_pool,
            tc.tile_pool(name="rdram", bufs=2, space="DRAM") as rd_pool,
        ):
            # All PSUM comes from two 2-bank pools: st (S-score tiles, and
            # short-lived V'/QK/out-projection group tiles) and cs (ctx
            # accumulators only; out-projection after phase C). Every st
            # allocation's awaited release (exp read / copy) is emitted in
            # the same block, so the in-order queues can never deadlock.
            def emit_v(t, alt=False):
                nonlocal ci
                ps = st_pool.tile([P, 1024], FP32, tag="st")
                for j in range(8):
                    nc.tensor.matmul(ps[:, 0:C], xts(j, t * P, P), wv_sb[:, j, :],
                                     start=(j == 0), stop=(j == 7))
                dst = vp[t].rearrange("p (h e) -> p h e", e=65)[:, :, 0:64]
                src = ps.rearrange("p (h e) -> p h e", e=64)[:, 0:8, :]
                if alt:
                    copy_px(ci, dst, src)
                    ci += 1
                else:
                    nc.vector.tensor_copy(dst, src)

            def emit_qk(dst, wsb, hp, s):
                nonlocal ci
                ps = st_pool.tile([P, 1024], FP32, tag="st")
                for j in range(8):
                    nc.tensor.matmul(ps[:, 0:SPAN],
                                     wsb[:, j, hp * P:(hp + 1) * P],
                                     xts(j, s * SPAN, SPAN),
                                     start=(j == 0), stop=(j == 7))
                copy_px(ci, dst[hp][:, s * SPAN:(s + 1) * SPAN], ps[:, 0:SPAN])
                ci += 1

            def emit_norm(hp, s, cs):
                # normalize: rows 0..63 / row 64 (ones-column rowsum).
                # Denominator row is bounced through DRAM to broadcast it
                # across partitions 0:64 (no PE or PSUM involvement), then
                # approx-reciprocal at base partition 0 and DVE multiply.
                qsl = slice(s * SPAN, (s + 1) * SPAN)
                rs = n_pool.tile([P, 1024], FP32, tag="rs")
                nc.vector.tensor_copy(rs[64:65, :], cs[64:65, :])
                rd = rd_pool.tile([1024], FP32, tag="rd")
                nc.sync.dma_start(rd[None, :], rs[64:65, :])
                rb = n_pool.tile([P, 1024], FP32, tag="rb")
                nc.sync.dma_start(rb[0:64, :],
                                  rd[None, :].to_broadcast((64, 1024)))
                rr = n_pool.tile([P, 1024], FP32, tag="rr")
                nc.vector.reciprocal_approx_fast(rr[0:64, :], rb[0:64, :])
                nc.vector.tensor_mul(ctxT[hp][0:64, qsl],
                                     cs[0:64, 0:512], rr[0:64, 0:512])
                tmpB = n_pool.tile([P, SPAN], BF16, tag="tmpB")
                nc.vector.tensor_mul(tmpB[0:64, :],
                                     cs[0:64, 512:1024], rr[0:64, 512:1024])
                nc.sync.dma_start(ctxT[hp][64:128, qsl], tmpB[0:64, :])

            def emit_d(s, qb, nhs, pool, alt=False):
                # output projection for q-block qb of span s, D-halves nhs;
                # contraction over all 4 head-pairs
                nonlocal ci
                qg = 4 * s + qb
                ps = pool.tile([P, 1024], FP32,
                               tag="st" if pool is st_pool else "cs")
                for nh in nhs:
                    for hp in range(4):
                        nc.tensor.matmul(
                            ps[:, nh * SPAN:(nh + 1) * SPAN],
                            ctxT[hp][:, qg * P:(qg + 1) * P],
                            wo_sb[:, hp, nh * SPAN:(nh + 1) * SPAN],
                            start=(hp == 0), stop=(hp == 3))
                c0, w = nhs[0] * SPAN, len(nhs) * SPAN
                ot = o_pool.tile([P, 1024], FP32, tag="osb")
                if alt and w == 1024:
                    # split copy across both engines; each half's output
                    # DMA issues as soon as its own copy lands
                    nc.scalar.copy(ot[:, 0:SPAN], ps[:, 0:SPAN])
                    nc.sync.dma_start(
                        out_d[qg * P:(qg + 1) * P, 0:SPAN], ot[:, 0:SPAN])
                    nc.vector.tensor_copy(ot[:, SPAN:1024], ps[:, SPAN:1024])
                    nc.sync.dma_start(
                        out_d[qg * P:(qg + 1) * P, SPAN:1024],
                        ot[:, SPAN:1024])
                    return
                if alt:
                    copy_px(ci, ot[:, c0:c0 + w], ps[:, c0:c0 + w])
                    ci += 1
                else:
                    nc.vector.tensor_copy(ot[:, c0:c0 + w], ps[:, c0:c0 + w])
                nc.sync.dma_start(
                    out_d[qg * P:(qg + 1) * P, c0:c0 + w], ot[:, c0:c0 + w])

            def emit_c(hp, s, fillers):
                # Software-pipelined emission: k-block kb's S matmuls and
                # exp are emitted BEFORE k-block kb-1's ctx matmuls, so the
                # in-order PE queue streams S(kb) while ScalarE runs
                # exp(kb-1). One filler (normalize of the previous chunk,
                # then V'/out-projection groups) is popped per k-block.
                hA, hB = 2 * hp, 2 * hp + 1
                cs = None
                nkb = 4 * s + 4
                pend = None          # (e, kb, d) awaiting ctx emission

                def emit_ctx(e, kb, d):
                    nonlocal cs
                    if cs is None:
                        cs = cs_pool.tile([P, 1024], FP32, tag="cs")
                    w = SPAN - 128 * d
                    co = 128 * d
                    nc.tensor.matmul(cs[0:65, co:SPAN],
                                     vp[kb][:, hA * 65:(hA + 1) * 65],
                                     e[:, 0:w],
                                     start=(kb == 0), stop=(kb == nkb - 1))
                    nc.tensor.matmul(cs[0:65, 512 + co:1024],
                                     vp[kb][:, hB * 65:(hB + 1) * 65],
                                     e[:, 512:512 + w],
                                     start=(kb == 0), stop=(kb == nkb - 1))

                for kb in range(nkb):
                    ksl = slice(kb * P, (kb + 1) * P)
                    d = max(0, kb - 4 * s)      # diagonal offset 0..3
                    q0 = s * SPAN + 128 * d     # valid q start
                    w = SPAN - 128 * d          # valid width
                    qsl = slice(q0, (s + 1) * SPAN)
                    st = st_pool.tile([P, 1024], FP32, tag="st")
                    st3 = st.rearrange("p (b q) -> p b q", b=2)[:, :, 0:w]
                    nc.tensor.matmul(st[:, 0:w], kt[hp][0:64, ksl],
                                     qt[hp][0:64, qsl],
                                     start=True, stop=True)
                    nc.tensor.matmul(st[:, 512:512 + w], kt[hp][64:128, ksl],
                                     qt[hp][64:128, qsl],
                                     start=True, stop=True)
                    e = e_pool.tile([P, 1024], BF16, tag="e")
                    e3 = e.rearrange("p (b q) -> p b q", b=2)[:, :, 0:w]
                    nc.scalar.activation(e3, st3, Exp, scale=0.125)
                    if kb >= 4 * s:
                        # staircase lives in the leading 128 cols only
                        e128 = e.rearrange("p (b q) -> p b q", b=2)[:, :, 0:P]
                        nc.vector.tensor_mul(
                            e128, e128,
                            mask_sb[:, None, :].to_broadcast((P, 2, P)))
                    if pend is not None:
                        if fillers:
                            fillers.pop(0)()
                        emit_ctx(*pend)
                    pend = (e, kb, d)
                while fillers:
                    fillers.pop(0)()
                emit_ctx(*pend)
                return cs

            # ---- Phase B runway: V'(0..3) and all Q^T/K^T ---------------
            for t in range(4):
                emit_v(t, alt=True)
            for hp in range(4):
                for s in range(NSPAN):
                    emit_qk(qt, wq_sb, hp, s)
            for hp in range(4):
                for s in range(NSPAN):
                    emit_qk(kt, wk_sb, hp, s)

            # ---- Phase C with interleaved V' / output projection --------
            # filler schedule (2 PE filler groups per chunk):
            #   span 0: V'(4..11)    span 1: V'(12..15) + D(0) full groups
            #   span 2: D(1) halves  span 3: D(2) halves
            # Each out-projection group D(s, qb) is scheduled at least one
            # full chunk after norm(3, s) so the DRAM-bounce normalize
            # chain (~6us) never stalls its ctxT reads.
            prev = None
            for s in range(NSPAN):
                for hp in range(4):
                    fillers = []
                    if prev is not None:
                        ph, ps_, pc = prev
                        fillers.append(
                            lambda ph=ph, ps_=ps_, pc=pc:
                                emit_norm(ph, ps_, pc))
                    if s == 0:
                        fillers.append(lambda t=4 + 2 * hp: emit_v(t))
                        fillers.append(lambda t=5 + 2 * hp: emit_v(t))
                    elif s == 1:
                        fillers.append(lambda t=12 + hp: emit_v(t))
                        if hp > 0:
                            fillers.append(
                                lambda hp=hp: emit_d(0, hp - 1, (0, 1), st_pool))
                    else:
                        if hp == 0:
                            fillers.append(
                                lambda s=s: emit_d(s - 2, 3, (0, 1), st_pool))
                        else:
                            fillers.append(
                                lambda s=s, hp=hp:
                                    emit_d(s - 1, hp - 1, (0, 1), st_pool))
                    cs = emit_c(hp, s, fillers)
                    prev = (hp, s, cs)
            # Tail: D(2,3) and the hp<3 partial sums of D(3,0/1) are
            # emitted BEFORE the final normalize, so the PE chews them
            # while the DRAM-bounce chain for chunk (3,3) completes
            # (ctxT tile deps are coarse, so anything reading ctxT[*]
            # emitted after that norm waits for it).
            fhp, fs, fcs = prev
            fqsl = slice(fs * SPAN, (fs + 1) * SPAN)
            nc.vector.tensor_copy(zden[64:65, :], fcs[64:65, :])
            fbc = st_pool.tile([P, 1024], FP32, tag="st")
            nc.tensor.matmul(fbc[0:64, 0:512], ones_sb[64:128, 0:64],
                             zden[64:128, 0:512], start=True, stop=True)
            nc.tensor.matmul(fbc[0:64, 512:1024], ones_sb[64:128, 0:64],
                             zden[64:128, 512:1024], start=True, stop=True)
            frr = n_pool.tile([P, 1024], FP32, tag="rr")
            nc.vector.reciprocal_approx_fast(frr[0:64, :], fbc[0:64, :])
            nc.vector.tensor_mul(ctxT[fhp][0:64, fqsl],
                                 fcs[0:64, 0:512], frr[0:64, 0:512])
            ftmp = n_pool.tile([P, SPAN], BF16, tag="tmpB")
            nc.vector.tensor_mul(ftmp[0:64, :],
                                 fcs[0:64, 512:1024], frr[0:64, 512:1024])
            nc.sync.dma_start(ctxT[fhp][64:128, fqsl], ftmp[0:64, :])
            emit_d(NSPAN - 2, 3, (0, 1), st_pool, alt=True)
            pd = []
            for qb in (0, 1):
                ps3 = cs_pool.tile([P, 1024], FP32, tag="cs")
                for nh in (0, 1):
                    for hp in range(3):
                        nc.tensor.matmul(
                            ps3[:, nh * SPAN:(nh + 1) * SPAN],
                            ctxT[hp][:, (12 + qb) * P:(13 + qb) * P],
                            wo_sb[:, hp, nh * SPAN:(nh + 1) * SPAN],
                            start=(hp == 0), stop=False)
                pd.append(ps3)
            for qb in (0, 1):
                ps3 = pd[qb]
                for nh in (0, 1):
                    nc.tensor.matmul(
                        ps3[:, nh * SPAN:(nh + 1) * SPAN],
                        ctxT[3][:, (12 + qb) * P:(13 + qb) * P],
                        wo_sb[:, 3, nh * SPAN:(nh + 1) * SPAN],
                        start=False, stop=True)
                ot = o_pool.tile([P, 1024], FP32, tag="osb")
                nc.scalar.copy(ot[:, 0:SPAN], ps3[:, 0:SPAN])
                nc.sync.dma_start(
                    out_d[(12 + qb) * P:(13 + qb) * P, 0:SPAN], ot[:, 0:SPAN])
                nc.vector.tensor_copy(ot[:, SPAN:1024], ps3[:, SPAN:1024])
                nc.sync.dma_start(
                    out_d[(12 + qb) * P:(13 + qb) * P, SPAN:1024],
                    ot[:, SPAN:1024])
            for qb in (2, 3):
                emit_d(NSPAN - 1, qb, (0, 1), cs_pool, alt=True)

    nc.compile()
    return nc


def _get_program():
    global _program
    if _program is None:
        _program = _build()
    return _program


def _make_mask():
    import ml_dtypes
    j = np.arange(P)[None, :]
    k = np.arange(P)[:, None]
    return np.where(j >= k, 1.0, 0.0).astype(ml_dtypes.bfloat16)


def _make_in_maps(x, Wq, Wk, Wv, Wo):
    import ml_dtypes
    bf16 = ml_dtypes.bfloat16
    mask = _make_mask()
    wq = np.asarray(Wq, np.float32).astype(bf16)
    wk = np.asarray(Wk, np.float32).astype(bf16)
    wv = np.asarray(Wv, np.float32).astype(bf16)
    wo = np.asarray(Wo, np.float32).astype(bf16)
    # x transposed on host: one [D, T] array per batch, shared by 2 cores
    xts = [np.ascontiguousarray(np.asarray(x[b], np.float32).astype(bf16).T)
           for b in range(x.shape[0])]
    in_maps = []
    for c in range(N_CORES):
        b, g = c // 2, c % 2
        cols = slice(g * C, (g + 1) * C)
        in_maps.append({
            "xt": xts[b],
            "wq": np.ascontiguousarray(wq[:, cols]),
            "wk": np.ascontiguousarray(wk[:, cols]),
            "wv": np.ascontiguousarray(wv[:, cols]),
            "wo": np.ascontiguousarray(wo[cols, :]),
            "mask": mask,
        })
    return in_maps


def _combine(results, bo, B):
    out = np.empty((B, T, D), dtype=np.float32)
    bo = np.asarray(bo, dtype=np.float32)
    for b in range(B):
        out[b] = results[2 * b]["out"] + results[2 * b + 1]["out"] + bo
    return out


def kernel(x, Wq, Wk, Wv, Wo, bo):
    x = np.asarray(x)
    nc = _get_program()
    in_maps = _make_in_maps(x, Wq, Wk, Wv, Wo)
    res = run_bass_kernel_spmd(nc, in_maps, core_ids=list(range(N_CORES)))
    return _combine(res.results, bo, x.shape[0])


def kernel_traced(x, Wq, Wk, Wv, Wo, bo):
    """Like kernel() but also returns the BassKernelResults (with
    exec_time_ns when NTFF tracing is available)."""
    x = np.asarray(x)
    nc = _get_program()
    in_maps = _make_in_maps(x, Wq, Wk, Wv, Wo)
    res = run_bass_kernel_spmd(nc, in_maps, core_ids=list(range(N_CORES)),
                               trace=True)
    return _combine(res.results, bo, x.shape[0]), res
